# revision 1
# baseline (speedup 1.0000x reference)
"""Multi-head attention block (QKV proj + softmax attention + out-proj +
residual + LayerNorm) on 8 TRN2 NeuronCores.

Sharding: core = (batch b, token-half g). Each core computes attention for
its 1024 query tokens over all 8 heads (K/V over the full 2048 tokens of its
batch are recomputed per pair — cheaper than cross-core collectives), then
the output projection, residual and LayerNorm for its token half. Outputs
are disjoint [1024, 1024] shards concatenated on the host.

Inputs are token-rotated per core on the host so that rows 0..1023 of the
per-core `x` are always that core's query tokens (softmax over k is
permutation-invariant, so K/V built from the rotated order are fine). The
host also pre-transposes x to d-major bf16, so the kernel needs no
on-device transposes of x.

Matmuls run in bf16 (PE full rate + fast weight load); accumulation is
fp32 in PSUM, softmax statistics and LayerNorm are fp32. The attention
k-chunk loop is software-pipelined: scores+exp for chunk kc+1 issue ahead
of the PV/ones matmuls of chunk kc so the ScalarE exp latency is hidden.
"""

import contextlib
import sys

if '/opt/trn_rl_repo' not in sys.path:
    sys.path.insert(0, '/opt/trn_rl_repo')

import ml_dtypes
import numpy as np

import concourse.bacc as bacc
import concourse.bass as bass
import concourse.bass_utils as bass_utils
import concourse.tile as tile
from concourse import mybir
from concourse.masks import make_identity

B, T, D, H = 4, 2048, 1024, 8
DH = 128            # head dim
TQ = T // 2         # query tokens per core
N_CORES = 8
DC = D // 128       # d-chunks of 128
KC = T // 128       # k-token chunks of 128
QC = TQ // 128      # q-token chunks of 128
EPS = 1e-5
ISCALE = 1.0 / float(np.sqrt(DH))
F32 = mybir.dt.float32
BF16 = mybir.dt.bfloat16
AF = mybir.ActivationFunctionType
ALU = mybir.AluOpType
BF = ml_dtypes.bfloat16


def _body(nc, tc, ap, es, apply_gb):
    xq, xbT, Wq, bq, Wk, bk, Wv, bv, Wo, gamma, beta, y = (
        ap['xq'], ap['xbT'], ap['Wq'], ap['bq'], ap['Wk'], ap['bk'],
        ap['Wv'], ap['bv'], ap['Wo'], ap['gamma'], ap['beta'], ap['y'])

    consts = es.enter_context(tc.tile_pool(name="consts", bufs=1))
    ctx_pool = es.enter_context(tc.tile_pool(name="ctx", bufs=1))
    xt_pool = es.enter_context(tc.tile_pool(name="xt", bufs=1))
    w_pool = es.enter_context(tc.tile_pool(name="w", bufs=6))
    kt_pool = es.enter_context(tc.tile_pool(name="kt", bufs=2))
    vt_pool = es.enter_context(tc.tile_pool(name="vt", bufs=2))
    v_pool = es.enter_context(tc.tile_pool(name="v", bufs=2))
    qt_pool = es.enter_context(tc.tile_pool(name="qt", bufs=2))
    pt_pool = es.enter_context(tc.tile_pool(name="pt", bufs=4))
    sums_pool = es.enter_context(tc.tile_pool(name="sums", bufs=2))
    wo_pool = es.enter_context(tc.tile_pool(name="wo", bufs=1))
    xr_pool = es.enter_context(tc.tile_pool(name="xr", bufs=2))
    y3_pool = es.enter_context(tc.tile_pool(name="y3", bufs=2))
    ln_pool = es.enter_context(tc.tile_pool(name="ln", bufs=4))

    # ---- constants -------------------------------------------------------
    ident = consts.tile([128, 128], BF16, tag="ident")
    make_identity(nc, ident)
    ones = consts.tile([128, 1], BF16, tag="ones")
    nc.vector.memset(ones, 1.0)
    eps_t = consts.tile([128, 1], F32, tag="eps")
    nc.vector.memset(eps_t, EPS)


    # partition-broadcast rows (per-feature vectors used on the free dim)
    def bcast128(name, src):
        t = consts.tile([128, D], F32, tag=name, name=name)
        src_b = bass.AP(tensor=src.tensor, offset=src.offset,
                        ap=[[0, 128]] + src.ap)
        nc.sync.dma_start(out=t, in_=src_b)
        return t


    ctx = [ctx_pool.tile([128, TQ], BF16, tag=f"ctx{h}", name=f"ctx{h}")
           for h in range(H)]

    # x^T (d-major) comes pre-transposed from the host: straight DMA
    # loads, split across two DMA paths to shorten the startup ramp
    xt = [xt_pool.tile([128, T], BF16, tag=f"xt{dc}", name=f"xt{dc}")
          for dc in range(DC)]
    for dc in range(DC):
        nc.sync.dma_start(out=xt[dc], in_=xbT[dc * 128:(dc + 1) * 128, :])

    # per-head bias layout: bias_t[p, h] = b[h*128 + p]
    bq_t = consts.tile([128, H], F32, tag="bq")
    bk_t = consts.tile([128, 4], F32, tag="bk")
    bv_t = consts.tile([128, 4], F32, tag="bv")
    nc.sync.dma_start(out=bq_t, in_=bq.rearrange("(h p) -> p h", p=128))
    nc.sync.dma_start(out=bk_t, in_=bk.rearrange("(h p) -> p h", p=128))
    nc.sync.dma_start(out=bv_t, in_=bv.rearrange("(h p) -> p h", p=128))

    # Wo blocks: prefetched mid phase 2 (read only in phase 3)
    wo_t = [wo_pool.tile([128, D], BF16, tag=f"wo{dc}", name=f"wo{dc}")
            for dc in range(DC)]

    # ---- phase 2: local K/V + pair AllGather, then per-head attention ----
    with contextlib.ExitStack() as es2:
        wk_psum = es2.enter_context(tc.tile_pool(name="wk_ps", bufs=4,
                                                 space="PSUM"))
        ctx_psum = es2.enter_context(tc.tile_pool(name="ctx_ps", bufs=1,
                                                  space="PSUM"))
        sum_psum = es2.enter_context(tc.tile_pool(name="sum_ps", bufs=1,
                                                  space="PSUM"))
        dram = es2.enter_context(tc.tile_pool(name="dram", bufs=1,
                                              space="DRAM"))

        def proj_nt(dst, w_b, bias_col, nt):
            nsl = slice(nt * 512, (nt + 1) * 512)
            pp = wk_psum.tile([128, 512], F32, tag="ps", name="pp")
            for dc in range(DC):
                nc.tensor.matmul(pp, w_b[dc], xt[dc][:, nsl],
                                 start=(dc == 0), stop=(dc == DC - 1))
            nc.vector.tensor_scalar(out=dst[:, nsl], in0=pp,
                                    scalar1=bias_col, scalar2=None,
                                    op0=ALU.add)

        # --- stage A: K/V for this core's 4 local heads (host-permuted
        # Wk/Wv pick the right actual heads), exchanged with the pair
        # partner via AllGather; group order makes k_all/v_all canonical
        # (entry hh = actual head hh) on both cores.
        k_send = dram.tile([4, 128, T], BF16, tag="k_send")
        v_send = dram.tile([4, 128, T], BF16, tag="v_send")
        k_all = dram.tile([H, 128, T], BF16, tag="k_all")
        v_all = dram.tile([H, 128, T], BF16, tag="v_all")

        def dma_w_blocks(W, hsl, tag):
            blocks = []
            for dc in range(DC):
                dsl = slice(dc * 128, (dc + 1) * 128)
                wt = w_pool.tile([128, 128], BF16, tag=tag, name="wb")
                nc.sync.dma_start(out=wt, in_=W[dsl, hsl])
                blocks.append(wt)
            return blocks

        for j in range(4):
            jsl = slice(j * 128, (j + 1) * 128)
            wk_b = dma_w_blocks(Wk, jsl, f"wk{j % 2}")
            ktl = kt_pool.tile([128, T], BF16, tag="kt", name=f"ktl{j}")
            for nt in range(T // 512):
                proj_nt(ktl, wk_b, bk_t[:, j:j + 1], nt)
            nc.sync.dma_start(out=k_send[j], in_=ktl)
        nc.gpsimd.collective_compute(
            "AllGather", mybir.AluOpType.bypass,
            ins=[k_send.opt()], outs=[k_all.opt()],
            replica_groups=[[0, 1], [2, 3], [4, 5], [6, 7]])

        for j in range(4):
            jsl = slice(j * 128, (j + 1) * 128)
            wv_b = dma_w_blocks(Wv, jsl, f"wv{j % 2}")
            vtl = vt_pool.tile([128, T], BF16, tag="vt", name=f"vtl{j}")
            for nt in range(T // 512):
                proj_nt(vtl, wv_b, bv_t[:, j:j + 1], nt)
            vl = v_pool.tile([128, KC, 128], BF16, tag="v", name=f"vl{j}")
            for kc in range(KC):
                tp = wk_psum.tile([128, 128], BF16, tag="ps", name="tpv")
                nc.tensor.transpose(tp, vtl[:, kc * 128:(kc + 1) * 128],
                                    ident)
                nc.vector.tensor_copy(out=vl[:, kc, :], in_=tp)
            nc.sync.dma_start(out=v_send[j],
                              in_=vl.rearrange("p a b -> p (a b)"))
        nc.gpsimd.collective_compute(
            "AllGather", mybir.AluOpType.bypass,
            ins=[v_send.opt()], outs=[v_all.opt()],
            replica_groups=[[0, 1], [2, 3], [4, 5], [6, 7]])

        # --- per-head state: Q projection tasks + K/V fetch from the
        # gathered buffers (canonical head order, uniform across cores)
        def load_kv(h):
            kt = kt_pool.tile([128, T], BF16, tag="ktg", name=f"kt{h}",
                              bufs=3)
            nc.sync.dma_start(out=kt, in_=k_all[h])
            v = v_pool.tile([128, KC, 128], BF16, tag="vg", name=f"v{h}",
                            bufs=3)
            nc.sync.dma_start(out=v.rearrange("p a b -> p (a b)"),
                              in_=v_all[h])
            return kt, v

        def make_head_tasks(h):
            wq_b = dma_w_blocks(Wq, slice(h * 128, (h + 1) * 128),
                                f"wq{h % 2}")
            qt = qt_pool.tile([128, TQ], BF16, tag="qt", name=f"qt{h}")
            tasks = [lambda nt=nt: proj_nt(qt, wq_b, bq_t[:, h:h + 1], nt)
                     for nt in range(TQ // 512)]
            return {'qt': qt, 'tasks': tasks}

        kv_tiles = {0: load_kv(0), 1: load_kv(1)}

        # prologue: head 0 Q projection runs un-interleaved
        head_cur = make_head_tasks(0)
        for t in head_cur['tasks']:
            t()
        head_cur['tasks'] = []

        for h in range(H):
            if h + 1 < H:
                head_next = make_head_tasks(h + 1)
            else:
                head_next = None
            if h + 2 < H:
                kv_tiles[h + 2] = load_kv(h + 2)
            if h == 1:
                gb = [bcast128("gamma_b", gamma), bcast128("beta_b", beta)] \
                    if apply_gb else None
            if h == 2:
                for dc in range(DC):
                    nc.sync.dma_start(out=wo_t[dc],
                                      in_=Wo[dc * 128:(dc + 1) * 128, :])

            kt, v = kv_tiles.pop(h)
            qt = head_cur['qt']
            bg = list(head_next['tasks']) if head_next else []
            bg_i = 0

            # attention, software-pipelined over k-chunks; next head's
            # projection tasks are drip-fed between chunks to keep the PE
            # busy while ScalarE works through the exps
            ctx_ps = ctx_psum.tile([128, TQ], F32, tag="ctx_ps")
            sum_ps = sum_psum.tile([1, TQ], F32, tag="sum_ps")

            def scores_exp(kc):
                ks = slice(kc * 128, (kc + 1) * 128)
                pt = pt_pool.tile([128, TQ], BF16, tag="pt", name="pt")
                for nq in range(TQ // 512):
                    nsl = slice(nq * 512, (nq + 1) * 512)
                    s_ps = wk_psum.tile([128, 512], F32, tag="ps",
                                        name="s_ps")
                    nc.tensor.matmul(s_ps, kt[:, ks], qt[:, nsl],
                                     start=True, stop=True)
                    nc.scalar.activation(out=pt[:, nsl], in_=s_ps,
                                         func=AF.Exp, scale=ISCALE)
                return pt

            pt_cur = scores_exp(0)
            for kc in range(KC):
                pt_next = scores_exp(kc + 1) if kc + 1 < KC else None
                for nq in range(TQ // 512):
                    nsl = slice(nq * 512, (nq + 1) * 512)
                    nc.tensor.matmul(ctx_ps[:, nsl], v[:, kc, :],
                                     pt_cur[:, nsl],
                                     start=(kc == 0), stop=(kc == KC - 1))
                for nq in range(TQ // 512):
                    nsl = slice(nq * 512, (nq + 1) * 512)
                    nc.tensor.matmul(sum_ps[:, nsl], ones, pt_cur[:, nsl],
                                     start=(kc == 0), stop=(kc == KC - 1))
                if bg_i < len(bg):
                    bg[bg_i]()
                    bg_i += 1
                pt_cur = pt_next
            while bg_i < len(bg):
                bg[bg_i]()
                bg_i += 1

            # free the PSUM accumulators fast; normalize off the
            # critical path (reciprocal + broadcast + in-place scale)
            nc.vector.tensor_copy(out=ctx[h], in_=ctx_ps)
            ssb = sums_pool.tile([1, TQ], F32, tag="ssb")
            nc.vector.tensor_copy(out=ssb, in_=sum_ps)
            rsum = sums_pool.tile([1, TQ], F32, tag="rsum")
            nc.vector.reciprocal_approx_fast(out=rsum, in_=ssb)
            rsum_b = sums_pool.tile([128, TQ], F32, tag="rsum_b")
            nc.gpsimd.partition_broadcast(rsum_b, rsum, channels=128)
            nc.vector.tensor_mul(out=ctx[h], in0=ctx[h], in1=rsum_b)

            head_cur = head_next

    # ---- phase 3: out-projection + residual + LayerNorm ------------------
    with tc.tile_pool(name="y_ps", bufs=2, space="PSUM") as y_psum:
        for qc in range(QC):
            qs = slice(qc * 128, (qc + 1) * 128)
            y_ps = y_psum.tile([128, D], F32, tag="y_ps")
            for no in range(D // 512):
                nsl = slice(no * 512, (no + 1) * 512)
                for dc in range(DC):
                    nc.tensor.matmul(y_ps[:, nsl], ctx[dc][:, qs],
                                     wo_t[dc][:, nsl],
                                     start=(dc == 0), stop=(dc == DC - 1))

            xr = xr_pool.tile([128, D], F32, tag="xr")
            nc.sync.dma_start(out=xr, in_=xq[qc * 128:(qc + 1) * 128, :])
            y1 = y3_pool.tile([128, D], F32, tag="y1")
            nc.vector.tensor_add(out=y1, in0=y_ps, in1=xr)  # resid (+bo)

            # LayerNorm over the feature dim
            stats = ln_pool.tile([128, 2, 6], F32, tag="stats")
            y1g = y1.rearrange("p (n f) -> p n f", f=512)
            nc.vector.bn_stats(out=stats[:, 0, :], in_=y1g[:, 0, :])
            nc.vector.bn_stats(out=stats[:, 1, :], in_=y1g[:, 1, :])
            mv = ln_pool.tile([128, 2], F32, tag="mv")
            nc.vector.bn_aggr(out=mv, in_=stats)
            std = ln_pool.tile([128, 1], F32, tag="std")
            nc.scalar.activation(out=std, in_=mv[:, 1:2], func=AF.Sqrt,
                                 bias=eps_t)
            rstd = ln_pool.tile([128, 1], F32, tag="rstd")
            nc.vector.reciprocal(out=rstd, in_=std)
            y2 = y3_pool.tile([128, D], F32, tag="y2")
            nc.vector.tensor_scalar(out=y2, in0=y1, scalar1=mv[:, 0:1],
                                    scalar2=rstd, op0=ALU.subtract,
                                    op1=ALU.mult)
            if apply_gb:
                nc.vector.tensor_mul(out=y2, in0=y2, in1=gb[0])
                nc.vector.tensor_add(out=y2, in0=y2, in1=gb[1])
            nc.sync.dma_start(out=y[qs, :], in_=y2)


def build(apply_gb=True):
    nc = bacc.Bacc("TRN2", target_bir_lowering=False, debug=False,
                   enable_asserts=False, num_devices=N_CORES)
    ap = {}
    ap['xq'] = nc.dram_tensor("xq", [TQ, D], F32, kind="ExternalInput").ap()
    ap['xbT'] = nc.dram_tensor("xbT", [D, T], BF16, kind="ExternalInput").ap()
    ap['Wq'] = nc.dram_tensor("Wq", [D, D], BF16, kind="ExternalInput").ap()
    ap['bq'] = nc.dram_tensor("bq", [D], F32, kind="ExternalInput").ap()
    ap['Wo'] = nc.dram_tensor("Wo", [D, D], BF16, kind="ExternalInput").ap()
    ap['Wk'] = nc.dram_tensor("Wk", [D, 512], BF16,
                              kind="ExternalInput").ap()
    ap['bk'] = nc.dram_tensor("bk", [512], F32, kind="ExternalInput").ap()
    ap['Wv'] = nc.dram_tensor("Wv", [D, 512], BF16,
                              kind="ExternalInput").ap()
    ap['bv'] = nc.dram_tensor("bv", [512], F32, kind="ExternalInput").ap()
    ap['gamma'] = nc.dram_tensor("gamma", [D], F32, kind="ExternalInput").ap()
    ap['beta'] = nc.dram_tensor("beta", [D], F32, kind="ExternalInput").ap()
    ap['y'] = nc.dram_tensor("y", [TQ, D], F32, kind="ExternalOutput").ap()

    with tile.TileContext(nc) as tc, contextlib.ExitStack() as es:
        _body(nc, tc, ap, es, apply_gb)
    nc.compile()
    return nc


def make_in_maps(inputs):
    """Per-core input maps; x token-rotated so q tokens come first."""
    f32 = {k: np.ascontiguousarray(np.asarray(v, dtype=np.float32))
           for k, v in inputs.items()}
    shared = {k: f32[k] for k in ('bq', 'gamma', 'beta')}
    for w in ('Wq', 'Wo'):
        shared[w] = np.ascontiguousarray(f32[w].astype(BF))
    wk_bf = f32['Wk'].astype(BF)
    wv_bf = f32['Wv'].astype(BF)
    x = f32['x']
    in_maps = []
    for core in range(N_CORES):
        b, g = divmod(core, 2)
        own = slice(512 * g, 512 * (g + 1))
        xr = np.roll(x[b], -TQ * g, axis=0)
        in_maps.append({'xq': np.ascontiguousarray(xr[:TQ] + f32['bo']),
                        'xbT': np.ascontiguousarray(xr.T.astype(BF)),
                        'Wk': np.ascontiguousarray(wk_bf[:, own]),
                        'bk': f32['bk'][own].copy(),
                        'Wv': np.ascontiguousarray(wv_bf[:, own]),
                        'bv': f32['bv'][own].copy(),
                        **shared})
    return in_maps


_NC = None
_NC_GB = None


def kernel(**inputs):
    global _NC, _NC_GB
    apply_gb = not (np.all(np.asarray(inputs['gamma']) == 1.0)
                    and np.all(np.asarray(inputs['beta']) == 0.0))
    if _NC is None or _NC_GB != apply_gb:
        _NC = build(apply_gb)
        _NC_GB = apply_gb
    in_maps = make_in_maps(inputs)
    res = bass_utils.run_bass_kernel_spmd(_NC, in_maps,
                                          core_ids=list(range(N_CORES)))
    out = np.empty((B, T, D), dtype=np.float32)
    for core in range(N_CORES):
        b, g = divmod(core, 2)
        out[b, TQ * g:TQ * (g + 1)] = res.results[core]['y']
    return out



# revision 10
# speedup vs baseline: 1.5811x; 1.5811x over previous
"""Multi-head attention block (QKV proj + softmax attention + out-proj +
residual + LayerNorm) on 8 TRN2 NeuronCores.

Sharding: core = (batch b, token-half g). Each core computes K/V for the
FULL 2048 tokens of its batch locally (all 8 heads) -- no collectives at
all -- and runs attention + out-proj + LayerNorm for its 1024 query
tokens. Host rotates tokens per core so the core's query tokens are
always columns 0..1023 (softmax over k is permutation-invariant).

Precision: weights and x are pre-quantized to fp8 (e4m3) on the host
(weights pre-scaled x32 for mantissa range); projection / PV / sum /
out-proj matmuls run in fp8 DoubleRow perf mode (256-deep contraction
per instruction = 2x bf16 rate). DoubleRow operands are host-packed so
every lhsT/rhs slice has its two k-planes contiguous (ISA restriction
's3_lw_dual_fp8_restrictions'). Scores (contraction 128) run plain fp8.
PSUM accumulation is fp32; softmax statistics and LayerNorm are fp32.

Bias algebra (host-folded): bk drops out of softmax entirely (adds a
per-query constant to every logit); bv's contribution to ctx is exactly
bv (softmax weights sum to 1) so bv@Wo + bo folds into the residual
term; only bq survives in-kernel (added to Q). The residual is
pre-scaled x2048 to absorb all fp8 weight scales -- LayerNorm is
scale-invariant so only eps needs adjusting.
"""

import contextlib
import sys

if '/opt/trn_rl_repo' not in sys.path:
    sys.path.insert(0, '/opt/trn_rl_repo')

import ml_dtypes
import numpy as np

import concourse.bacc as bacc
import concourse.bass as bass
import concourse.bass_utils as bass_utils
import concourse.tile as tile
from concourse import mybir

B, T, D, H = 4, 2048, 1024, 8
DH = 128
TQ = T // 2
N_CORES = 8
NP = 4              # d-chunk pairs (contraction 1024 = 4 x 256)
CP = 8              # k-chunk pairs (2048 = 8 x 256)
NQ = 2              # 512-wide q slices per core
QC = TQ // 128
EPS = 1e-5
WS = 32.0           # fp8 weight pre-scale
XQS = 2048.0        # residual pre-scale (= ctx-scale 64 x Wo-scale 32)
SC_EXP = 1.0 / (WS * WS * float(np.sqrt(DH)))
EPS_S = XQS * XQS * EPS
F32 = mybir.dt.float32
BF16 = mybir.dt.bfloat16
FP8 = mybir.dt.float8e4
AF = mybir.ActivationFunctionType
ALU = mybir.AluOpType
DR = mybir.MatmulPerfMode.DoubleRow
BF = ml_dtypes.bfloat16
F8 = ml_dtypes.float8_e4m3


def _body(nc, tc, ap, es, apply_gb):
    xtm_d, xts_d = ap['xtm'], ap['xts']
    wq_d, wk_d, wv_d, wo_d = ap['wq'], ap['wk'], ap['wv'], ap['wo']
    bq_d, xq_d, gamma, beta, y = (ap['bq'], ap['xq'], ap['gamma'],
                                  ap['beta'], ap['y'])

    consts = es.enter_context(tc.tile_pool(name="consts", bufs=1))
    xt_pool = es.enter_context(tc.tile_pool(name="xt", bufs=1))
    w_pool = es.enter_context(tc.tile_pool(name="w", bufs=1))
    kts_pool = es.enter_context(tc.tile_pool(name="kts", bufs=1))
    qts_pool = es.enter_context(tc.tile_pool(name="qts", bufs=1))
    vp_pool = es.enter_context(tc.tile_pool(name="vp", bufs=1))
    ctx4_pool = es.enter_context(tc.tile_pool(name="ctx4", bufs=1))
    pt_pool = es.enter_context(tc.tile_pool(name="pt", bufs=12))
    sums_pool = es.enter_context(tc.tile_pool(name="sums", bufs=3))
    xr_pool = es.enter_context(tc.tile_pool(name="xr", bufs=4))
    y2_pool = es.enter_context(tc.tile_pool(name="y2", bufs=2))
    ln_pool = es.enter_context(tc.tile_pool(name="ln", bufs=4))
    ps512 = es.enter_context(tc.tile_pool(name="ps512", bufs=4,
                                          space="PSUM"))

    # ---- constants & weights ---------------------------------------------
    # dual-fp8 LDWEIGHTS needs the two k-planes >=16B apart; pad to 16
    ones2_t = consts.tile([128, 2, 16], FP8, tag="ones2")
    nc.vector.memset(ones2_t, 0.5)
    ones2 = ones2_t[:, :, 0:1]
    eps_t = consts.tile([128, 1], F32, tag="eps")
    nc.vector.memset(eps_t, EPS_S)
    bq_t = consts.tile([128, H], F32, tag="bq")
    nc.sync.dma_start(out=bq_t, in_=bq_d)

    def dma4(dst, src):
        nc.sync.dma_start(out=dst.rearrange("p a b c -> p (a b c)"),
                          in_=src.rearrange("p a b c -> p (a b c)"))

    # x^T in two DoubleRow-friendly packings: 512-wide windows (moving
    # side of K/Q proj) and 128-wide chunks (stationary side of V proj)
    xtm, xts = [], []
    for p in range(NP):
        tm = xt_pool.tile([128, 4, 2, 512], FP8, tag=f"xtm{p}",
                          name=f"xtm{p}")
        dma4(tm, xtm_d[p])
        xtm.append(tm)
        tsx = xt_pool.tile([128, 16, 2, 128], FP8, tag=f"xts{p}",
                           name=f"xts{p}")
        dma4(tsx, xts_d[p])
        xts.append(tsx)

    def load_w(w_d, nm, shape):
        ts = []
        for p in range(NP):
            t = w_pool.tile(shape, FP8, tag=f"{nm}{p}", name=f"{nm}{p}")
            dma4(t, w_d[p])
            ts.append(t)
        return ts

    wk_t = load_w(wk_d, "wk", [128, H, 2, 128])
    wq_t = load_w(wq_d, "wq", [128, H, 2, 128])
    wv_t = load_w(wv_d, "wv", [128, 2, 2, 512])
    wo_t = load_w(wo_d, "wo", [128, 2, 2, 512])

    gb = None
    if apply_gb:
        def bcast128(name, src):
            t = consts.tile([128, D], F32, tag=name, name=name)
            src_b = bass.AP(tensor=src.tensor, offset=src.offset,
                            ap=[[0, 128]] + src.ap)
            nc.sync.dma_start(out=t, in_=src_b)
            return t
        gb = [bcast128("gamma_b", gamma), bcast128("beta_b", beta)]

    kts = [kts_pool.tile([128, T], FP8, tag=f"kts{h}", name=f"kts{h}")
           for h in range(H)]
    qts = [qts_pool.tile([128, TQ], FP8, tag=f"qts{h}", name=f"qts{h}")
           for h in range(H)]
    vp = [vp_pool.tile([128, H, 2, 128], FP8, tag=f"vp{c}", name=f"vp{c}")
          for c in range(CP)]
    ctx4 = [ctx4_pool.tile([128, QC, 2, 128], FP8, tag=f"ctx{p}",
                           name=f"ctx{p}")
            for p in range(NP)]

    # ---- projection helpers ----------------------------------------------
    def proj_unit_K(h, nt):
        nsl = slice(nt * 512, (nt + 1) * 512)
        pp = ps512.tile([128, 512], F32, tag="ps", name="ppk")
        for p in range(NP):
            nc.tensor.matmul(pp, wk_t[p][:, h], xtm[p][:, nt],
                             start=(p == 0), stop=(p == NP - 1),
                             perf_mode=DR)
        nc.vector.tensor_copy(out=kts[h][:, nsl], in_=pp)

    def proj_unit_Q(h, nt):
        nsl = slice(nt * 512, (nt + 1) * 512)
        pp = ps512.tile([128, 512], F32, tag="ps", name="ppq")
        for p in range(NP):
            nc.tensor.matmul(pp, wq_t[p][:, h], xtm[p][:, nt],
                             start=(p == 0), stop=(p == NP - 1),
                             perf_mode=DR)
        nc.vector.tensor_scalar(out=qts[h][:, nsl], in0=pp,
                                scalar1=bq_t[:, h:h + 1], scalar2=None,
                                op0=ALU.add)

    def head_tasks(h):
        return ([lambda nt=nt: proj_unit_K(h, nt) for nt in range(4)]
                + [lambda nt=nt: proj_unit_Q(h, nt) for nt in range(2)])

    # ---- phase A: head-0 projections -------------------------------------
    for t in head_tasks(0):
        t()

    # score/exp issue machinery (runs ahead of the PV consumer)
    steps = [(h, nq, cp) for h in range(H) for nq in range(NQ)
             for cp in range(CP)]
    pt_q = {}
    cursor = [0]

    def issue_scores():
        i = cursor[0]
        h, nq, cp = steps[i]
        nsl = slice(nq * 512, (nq + 1) * 512)
        pt = pt_pool.tile([128, 2, 512], FP8, tag="pt", name="pt")
        sps = []
        for j in range(2):
            kc = cp * 2 + j
            s_ps = ps512.tile([128, 512], F32, tag="ps", name="s_ps")
            nc.tensor.matmul(s_ps, kts[h][:, kc * 128:(kc + 1) * 128],
                             qts[h][:, nsl], start=True, stop=True)
            sps.append(s_ps)
        for j in range(2):
            nc.scalar.activation(out=pt[:, j, :], in_=sps[j], func=AF.Exp,
                                 scale=SC_EXP)
        pt_q[i] = pt
        cursor[0] += 1

    # ---- phase B: V-proj interleaved with head-0 scores ------------------
    with tc.tile_pool(name="psv", bufs=2, space="PSUM") as psV:
        for c in range(2 * CP):
            ppv = psV.tile([128, D], F32, tag="psv", name="ppv")
            for n2 in range(2):
                n2sl = slice(n2 * 512, (n2 + 1) * 512)
                for p in range(NP):
                    nc.tensor.matmul(ppv[:, n2sl], xts[p][:, c],
                                     wv_t[p][:, n2],
                                     start=(p == 0), stop=(p == NP - 1),
                                     perf_mode=DR)
            nc.vector.tensor_copy(
                out=vp[c // 2][:, :, c % 2, :],
                in_=ppv.rearrange("p (h m) -> p h m", m=128))
            if cursor[0] < 10:
                issue_scores()

    # ---- phase C: attention ----------------------------------------------
    # residual prefetch (needed in phase D; DMA has slack here)
    xr = {}

    def fetch_xr(qc):
        t = xr_pool.tile([128, D], F32, tag="xr", name="xr")
        nc.sync.dma_start(out=t, in_=xq_d[qc * 128:(qc + 1) * 128, :])
        xr[qc] = t

    for qc in range(4):
        fetch_xr(qc)

    def normalize(h, nq, ctx_ps, sum_ps):
        ssb = sums_pool.tile([1, 512], F32, tag="ssb", name="ssb")
        nc.vector.tensor_copy(out=ssb, in_=sum_ps)
        rsum = sums_pool.tile([1, 512], F32, tag="rsum", name="rsum")
        nc.vector.reciprocal_approx_fast(out=rsum, in_=ssb)
        rsum_b = sums_pool.tile([128, 512], F32, tag="rsum_b",
                                name="rsum_b")
        nc.gpsimd.partition_broadcast(rsum_b, rsum, channels=128)
        nc.vector.tensor_mul(
            out=ctx4[h // 2][:, 4 * nq:4 * nq + 4, h % 2, :],
            in0=ctx_ps.rearrange("p (a b) -> p a b", b=128),
            in1=rsum_b.rearrange("p (a b) -> p a b", b=128))

    with tc.tile_pool(name="ctxps", bufs=2, space="PSUM") as ctx_pool, \
         tc.tile_pool(name="sumps", bufs=2, space="PSUM") as sum_pool:
        drip = {h: head_tasks(h) for h in range(1, H)}
        ctx_cur = sum_cur = None
        for i, (h, nq, cp) in enumerate(steps):
            while cursor[0] <= min(i + 2, len(steps) - 1):
                issue_scores()
            if cp == 0:
                ctx_cur = ctx_pool.tile([128, 512], F32, tag="ctx",
                                        name="ctx_ps")
                sum_cur = sum_pool.tile([1, 512], F32, tag="sum",
                                        name="sum_ps")
            pt = pt_q.pop(i)
            nc.tensor.matmul(ctx_cur, vp[cp][:, h], pt,
                             start=(cp == 0), stop=(cp == CP - 1),
                             perf_mode=DR)
            nc.tensor.matmul(sum_cur, ones2, pt,
                             start=(cp == 0), stop=(cp == CP - 1),
                             perf_mode=DR)
            if h + 1 < H and drip[h + 1]:
                drip[h + 1].pop(0)()
            if cp == CP - 1:
                normalize(h, nq, ctx_cur, sum_cur)

    # ---- phase D: out-projection + residual + LayerNorm ------------------
    with tc.tile_pool(name="y_ps", bufs=2, space="PSUM") as y_psum:
        for qc in range(QC):
            if qc + 4 < QC:
                fetch_xr(qc + 4)
            qs = slice(qc * 128, (qc + 1) * 128)
            y_ps = y_psum.tile([128, D], F32, tag="y_ps", name="y_ps")
            for n2 in range(2):
                n2sl = slice(n2 * 512, (n2 + 1) * 512)
                for p in range(NP):
                    nc.tensor.matmul(y_ps[:, n2sl], ctx4[p][:, qc],
                                     wo_t[p][:, n2],
                                     start=(p == 0), stop=(p == NP - 1),
                                     perf_mode=DR)
            y1 = y2_pool.tile([128, D], F32, tag="y1", name="y1")
            nc.vector.tensor_add(out=y1, in0=y_ps, in1=xr.pop(qc))

            stats = ln_pool.tile([128, 2, 6], F32, tag="stats",
                                 name="stats")
            y1g = y1.rearrange("p (n f) -> p n f", f=512)
            nc.vector.bn_stats(out=stats[:, 0, :], in_=y1g[:, 0, :])
            nc.vector.bn_stats(out=stats[:, 1, :], in_=y1g[:, 1, :])
            mv = ln_pool.tile([128, 2], F32, tag="mv", name="mv")
            nc.vector.bn_aggr(out=mv, in_=stats)
            std = ln_pool.tile([128, 1], F32, tag="std", name="std")
            nc.scalar.activation(out=std, in_=mv[:, 1:2], func=AF.Sqrt,
                                 bias=eps_t)
            rstd = ln_pool.tile([128, 1], F32, tag="rstd", name="rstd")
            nc.vector.reciprocal(out=rstd, in_=std)
            y2 = y2_pool.tile([128, D], F32, tag="y2", name="y2")
            nc.vector.tensor_scalar(out=y2, in0=y1, scalar1=mv[:, 0:1],
                                    scalar2=rstd, op0=ALU.subtract,
                                    op1=ALU.mult)
            if apply_gb:
                nc.vector.tensor_mul(out=y2, in0=y2, in1=gb[0])
                nc.vector.tensor_add(out=y2, in0=y2, in1=gb[1])
            nc.sync.dma_start(out=y[qs, :], in_=y2)


def build(apply_gb=True):
    nc = bacc.Bacc("TRN2", target_bir_lowering=False, debug=False,
                   enable_asserts=False, num_devices=N_CORES)
    ap = {}
    ap['xtm'] = nc.dram_tensor("xtm", [NP, 128, 4, 2, 512], FP8,
                               kind="ExternalInput").ap()
    ap['xts'] = nc.dram_tensor("xts", [NP, 128, 16, 2, 128], FP8,
                               kind="ExternalInput").ap()
    for nm in ('wq', 'wk'):
        ap[nm] = nc.dram_tensor(nm, [NP, 128, H, 2, 128], FP8,
                                kind="ExternalInput").ap()
    for nm in ('wv', 'wo'):
        ap[nm] = nc.dram_tensor(nm, [NP, 128, 2, 2, 512], FP8,
                                kind="ExternalInput").ap()
    ap['bq'] = nc.dram_tensor("bq", [128, H], F32, kind="ExternalInput").ap()
    ap['xq'] = nc.dram_tensor("xq", [TQ, D], F32, kind="ExternalInput").ap()
    ap['gamma'] = nc.dram_tensor("gamma", [D], F32,
                                 kind="ExternalInput").ap()
    ap['beta'] = nc.dram_tensor("beta", [D], F32, kind="ExternalInput").ap()
    ap['y'] = nc.dram_tensor("y", [TQ, D], F32, kind="ExternalOutput").ap()

    with tile.TileContext(nc) as tc, contextlib.ExitStack() as es:
        _body(nc, tc, ap, es, apply_gb)
    nc.compile()
    return nc


def _pack_pairs(w8, inner):
    """[D, N] fp8 -> [NP, 128, N//inner, 2, inner]: row (2p+j)*128+r,
    col (o*inner+m) lands at [p, r, o, j, m] (k-plane pairs contiguous)."""
    n = w8.shape[1]
    return np.ascontiguousarray(
        w8.reshape(NP, 2, 128, n // inner, inner).transpose(0, 2, 3, 1, 4))


def make_in_maps(inputs):
    """Per-core input maps; x token-rotated so q tokens come first."""
    f32 = {k: np.asarray(v, dtype=np.float32) for k, v in inputs.items()}

    def w8(nm):
        return (f32[nm] * WS).astype(F8)

    shared = {
        'wq': _pack_pairs(w8('Wq'), 128),
        'wk': _pack_pairs(w8('Wk'), 128),
        'wv': _pack_pairs(w8('Wv'), 512),
        'wo': _pack_pairs(w8('Wo'), 512),
        'bq': np.ascontiguousarray(
            (WS * f32['bq']).reshape(H, 128).T.astype(np.float32)),
        'gamma': f32['gamma'],
        'beta': f32['beta'],
    }
    resid_c = f32['bo'] + f32['bv'] @ f32['Wo']
    x = f32['x']
    in_maps = []
    for core in range(N_CORES):
        b, g = divmod(core, 2)
        xr = np.roll(x[b], -TQ * g, axis=0)
        xt8 = xr.T.astype(F8)  # [D, T]
        in_maps.append({
            'xtm': _pack_pairs(xt8, 512),
            'xts': _pack_pairs(xt8, 128),
            'xq': np.ascontiguousarray(XQS * (xr[:TQ] + resid_c)),
            **shared})
    return in_maps


_NC = None
_NC_GB = None


def kernel(**inputs):
    global _NC, _NC_GB
    apply_gb = not (np.all(np.asarray(inputs['gamma']) == 1.0)
                    and np.all(np.asarray(inputs['beta']) == 0.0))
    if _NC is None or _NC_GB != apply_gb:
        _NC = build(apply_gb)
        _NC_GB = apply_gb
    in_maps = make_in_maps(inputs)
    res = bass_utils.run_bass_kernel_spmd(_NC, in_maps,
                                          core_ids=list(range(N_CORES)))
    out = np.empty((B, T, D), dtype=np.float32)
    for core in range(N_CORES):
        b, g = divmod(core, 2)
        out[b, TQ * g:TQ * (g + 1)] = res.results[core]['y']
    return out


# revision 11
# speedup vs baseline: 1.6768x; 1.0605x over previous
"""Multi-head attention block (QKV proj + softmax attention + out-proj +
residual + LayerNorm) on 8 TRN2 NeuronCores.

Sharding: core = (batch b, token-half g). Each core computes K/V for the
FULL 2048 tokens of its batch locally (all 8 heads) -- no collectives at
all -- and runs attention + out-proj + LayerNorm for its 1024 query
tokens. Host rotates tokens per core so the core's query tokens are
always columns 0..1023 (softmax over k is permutation-invariant).

Precision: weights and x are pre-quantized to fp8 (e4m3) on the host
(weights pre-scaled x32 for mantissa range); projection / PV / sum /
out-proj matmuls run in fp8 DoubleRow perf mode (256-deep contraction
per instruction = 2x bf16 rate). DoubleRow operand slices keep their two
k-planes >=16B apart (ISA rule 's3_lw_dual_fp8_restrictions'). Scores
(contraction 128) run plain fp8. PSUM accumulation is fp32; softmax
statistics and LayerNorm are fp32.

Schedule: q-slice (nq) outer, head inner. During the nq=0 pass the
next heads' K/Q projections drip between attention steps; during the
nq=1 pass the out-projection + LayerNorm for q-chunks 0..3 drips in,
so only half the epilogue remains after the last attention step.

Bias algebra (host-folded): bk drops out of softmax entirely (adds a
per-query constant to every logit); bv's contribution to ctx is exactly
bv (softmax weights sum to 1) so bv@Wo + bo folds into the residual
term; only bq survives in-kernel (added to Q). The residual is
pre-scaled x2048 to absorb all fp8 weight scales -- LayerNorm is
scale-invariant so only eps needs adjusting.
"""

import contextlib
import sys

if '/opt/trn_rl_repo' not in sys.path:
    sys.path.insert(0, '/opt/trn_rl_repo')

import ml_dtypes
import numpy as np

import concourse.bacc as bacc
import concourse.bass as bass
import concourse.bass_utils as bass_utils
import concourse.tile as tile
from concourse import mybir

B, T, D, H = 4, 2048, 1024, 8
DH = 128
TQ = T // 2
N_CORES = 8
NP = 4              # d-chunk pairs (contraction 1024 = 4 x 256)
CP = 8              # k-chunk pairs (2048 = 8 x 256)
NQ = 2              # 512-wide q slices per core
QC = TQ // 128
EPS = 1e-5
WS = 32.0           # fp8 weight pre-scale
XQS = 2048.0        # residual pre-scale (= ctx-scale 64 x Wo-scale 32)
SC_EXP = 1.0 / (WS * WS * float(np.sqrt(DH)))
EPS_S = XQS * XQS * EPS
F32 = mybir.dt.float32
BF16 = mybir.dt.bfloat16
FP8 = mybir.dt.float8e4
AF = mybir.ActivationFunctionType
ALU = mybir.AluOpType
DR = mybir.MatmulPerfMode.DoubleRow
BF = ml_dtypes.bfloat16
F8 = ml_dtypes.float8_e4m3


def _body(nc, tc, ap, es, apply_gb):
    xtm_d = ap['xtm']
    wq_d, wk_d, wv_d, wo_d = ap['wq'], ap['wk'], ap['wv'], ap['wo']
    bq_d, xq_d, gamma, beta, y = (ap['bq'], ap['xq'], ap['gamma'],
                                  ap['beta'], ap['y'])

    consts = es.enter_context(tc.tile_pool(name="consts", bufs=1))
    xt_pool = es.enter_context(tc.tile_pool(name="xt", bufs=1))
    w_pool = es.enter_context(tc.tile_pool(name="w", bufs=1))
    kts_pool = es.enter_context(tc.tile_pool(name="kts", bufs=1))
    qts_pool = es.enter_context(tc.tile_pool(name="qts", bufs=1))
    vp_pool = es.enter_context(tc.tile_pool(name="vp", bufs=1))
    ctx4_pool = es.enter_context(tc.tile_pool(name="ctx4", bufs=1))
    pt_pool = es.enter_context(tc.tile_pool(name="pt", bufs=12))
    sums_pool = es.enter_context(tc.tile_pool(name="sums", bufs=3))
    xr_pool = es.enter_context(tc.tile_pool(name="xr", bufs=4))
    y2_pool = es.enter_context(tc.tile_pool(name="y2", bufs=2))
    ln_pool = es.enter_context(tc.tile_pool(name="ln", bufs=4))
    ps512 = es.enter_context(tc.tile_pool(name="ps512", bufs=4,
                                          space="PSUM"))

    # ---- constants & weights (DMA order = need order) --------------------
    # dual-fp8 LDWEIGHTS needs the two k-planes >=16B apart; pad to 16
    ones2_t = consts.tile([128, 2, 16], FP8, tag="ones2")
    nc.vector.memset(ones2_t, 0.5)
    ones2 = ones2_t[:, :, 0:1]
    eps_t = consts.tile([128, 1], F32, tag="eps")
    nc.vector.memset(eps_t, EPS_S)
    bq_t = consts.tile([128, H], F32, tag="bq")
    nc.sync.dma_start(out=bq_t, in_=bq_d)

    def dma4(dst, src):
        nc.sync.dma_start(out=dst.rearrange("p a b c -> p (a b c)"),
                          in_=src.rearrange("p a b c -> p (a b c)"))

    def load_w(w_d, nm, shape):
        ts = []
        for p in range(NP):
            t = w_pool.tile(shape, FP8, tag=f"{nm}{p}", name=f"{nm}{p}")
            dma4(t, w_d[p])
            ts.append(t)
        return ts

    wk_t = load_w(wk_d, "wk", [128, H, 2, 128])
    xtm = []
    for p in range(NP):
        tm = xt_pool.tile([128, 4, 2, 512], FP8, tag=f"xtm{p}",
                          name=f"xtm{p}")
        dma4(tm, xtm_d[p])
        xtm.append(tm)
    wv_t = load_w(wv_d, "wv", [128, 2, 2, 512])
    wq_t = load_w(wq_d, "wq", [128, H, 2, 128])
    wo_t = load_w(wo_d, "wo", [128, 2, 2, 512])

    def xts(p, c):
        """V-proj stationary: [128, 2, 128] view of token chunk c."""
        return xtm[p][:, c // 4, :, (c % 4) * 128:(c % 4 + 1) * 128]

    gb = None
    if apply_gb:
        def bcast128(name, src):
            t = consts.tile([128, D], F32, tag=name, name=name)
            src_b = bass.AP(tensor=src.tensor, offset=src.offset,
                            ap=[[0, 128]] + src.ap)
            nc.sync.dma_start(out=t, in_=src_b)
            return t
        gb = [bcast128("gamma_b", gamma), bcast128("beta_b", beta)]

    kts = [kts_pool.tile([128, T], FP8, tag=f"kts{h}", name=f"kts{h}")
           for h in range(H)]
    qts = [qts_pool.tile([128, TQ], FP8, tag=f"qts{h}", name=f"qts{h}")
           for h in range(H)]
    vp = [vp_pool.tile([128, H, 2, 128], FP8, tag=f"vp{c}", name=f"vp{c}")
          for c in range(CP)]
    ctx4 = [ctx4_pool.tile([128, QC, 2, 128], FP8, tag=f"ctx{p}",
                           name=f"ctx{p}")
            for p in range(NP)]

    # ---- projection helpers ----------------------------------------------
    def proj_unit_K(h, nt):
        nsl = slice(nt * 512, (nt + 1) * 512)
        pp = ps512.tile([128, 512], F32, tag="ps", name="ppk")
        for p in range(NP):
            nc.tensor.matmul(pp, wk_t[p][:, h], xtm[p][:, nt],
                             start=(p == 0), stop=(p == NP - 1),
                             perf_mode=DR)
        nc.vector.tensor_copy(out=kts[h][:, nsl], in_=pp)

    def proj_unit_Q(h, nt):
        nsl = slice(nt * 512, (nt + 1) * 512)
        pp = ps512.tile([128, 512], F32, tag="ps", name="ppq")
        for p in range(NP):
            nc.tensor.matmul(pp, wq_t[p][:, h], xtm[p][:, nt],
                             start=(p == 0), stop=(p == NP - 1),
                             perf_mode=DR)
        nc.vector.tensor_scalar(out=qts[h][:, nsl], in0=pp,
                                scalar1=bq_t[:, h:h + 1], scalar2=None,
                                op0=ALU.add)

    def head_tasks(h):
        return ([lambda nt=nt: proj_unit_K(h, nt) for nt in range(4)]
                + [lambda nt=nt: proj_unit_Q(h, nt) for nt in range(2)])

    # ---- phase A: head-0 projections -------------------------------------
    for t in head_tasks(0):
        t()

    # score/exp issue machinery (runs ahead of the PV consumer)
    steps = [(nq, h, cp) for nq in range(NQ) for h in range(H)
             for cp in range(CP)]
    pt_q = {}
    cursor = [0]

    def issue_scores():
        i = cursor[0]
        nq, h, cp = steps[i]
        nsl = slice(nq * 512, (nq + 1) * 512)
        pt = pt_pool.tile([128, 2, 512], FP8, tag="pt", name="pt")
        sps = []
        for j in range(2):
            kc = cp * 2 + j
            s_ps = ps512.tile([128, 512], F32, tag="ps", name="s_ps")
            nc.tensor.matmul(s_ps, kts[h][:, kc * 128:(kc + 1) * 128],
                             qts[h][:, nsl], start=True, stop=True)
            sps.append(s_ps)
        for j in range(2):
            nc.scalar.activation(out=pt[:, j, :], in_=sps[j], func=AF.Exp,
                                 scale=SC_EXP)
        pt_q[i] = pt
        cursor[0] += 1

    # ---- phase B: V-proj interleaved with head-0 scores ------------------
    with tc.tile_pool(name="psv", bufs=2, space="PSUM") as psV:
        for c in range(2 * CP):
            ppv = psV.tile([128, D], F32, tag="psv", name="ppv")
            for n2 in range(2):
                n2sl = slice(n2 * 512, (n2 + 1) * 512)
                for p in range(NP):
                    nc.tensor.matmul(ppv[:, n2sl], xts(p, c),
                                     wv_t[p][:, n2],
                                     start=(p == 0), stop=(p == NP - 1),
                                     perf_mode=DR)
            nc.vector.tensor_copy(
                out=vp[c // 2][:, :, c % 2, :],
                in_=ppv.rearrange("p (h m) -> p h m", m=128))
            if cursor[0] < CP:
                issue_scores()

    # ---- phase C/D machinery ---------------------------------------------
    xr = {}

    def fetch_xr(qc):
        t = xr_pool.tile([128, D], F32, tag="xr", name="xr")
        nc.sync.dma_start(out=t, in_=xq_d[qc * 128:(qc + 1) * 128, :])
        xr[qc] = t

    for qc in range(4):
        fetch_xr(qc)

    def normalize(nq, h, ctx_ps, sum_ps):
        rsum = sums_pool.tile([1, 512], F32, tag="rsum", name="rsum")
        nc.vector.reciprocal_approx_fast(out=rsum, in_=sum_ps)
        rsum_b = sums_pool.tile([128, 512], F32, tag="rsum_b",
                                name="rsum_b")
        nc.gpsimd.partition_broadcast(rsum_b, rsum, channels=128)
        nc.vector.tensor_mul(
            out=ctx4[h // 2][:, 4 * nq:4 * nq + 4, h % 2, :],
            in0=ctx_ps.rearrange("p (a b) -> p a b", b=128),
            in1=rsum_b.rearrange("p (a b) -> p a b", b=128))

    def do_outproj(qc):
        if qc + 4 < QC:
            fetch_xr(qc + 4)
        qs = slice(qc * 128, (qc + 1) * 128)
        y1 = y2_pool.tile([128, D], F32, tag="y1", name="y1")
        for n2 in range(2):
            n2sl = slice(n2 * 512, (n2 + 1) * 512)
            pp = ps512.tile([128, 512], F32, tag="ps", name="ppo")
            for p in range(NP):
                nc.tensor.matmul(pp, ctx4[p][:, qc], wo_t[p][:, n2],
                                 start=(p == 0), stop=(p == NP - 1),
                                 perf_mode=DR)
            nc.vector.tensor_add(out=y1[:, n2sl], in0=pp,
                                 in1=xr[qc][:, n2sl])
        xr.pop(qc)

        stats = ln_pool.tile([128, 2, 6], F32, tag="stats", name="stats")
        y1g = y1.rearrange("p (n f) -> p n f", f=512)
        nc.vector.bn_stats(out=stats[:, 0, :], in_=y1g[:, 0, :])
        nc.vector.bn_stats(out=stats[:, 1, :], in_=y1g[:, 1, :])
        mv = ln_pool.tile([128, 2], F32, tag="mv", name="mv")
        nc.vector.bn_aggr(out=mv, in_=stats)
        std = ln_pool.tile([128, 1], F32, tag="std", name="std")
        nc.scalar.activation(out=std, in_=mv[:, 1:2], func=AF.Sqrt,
                             bias=eps_t)
        rstd = ln_pool.tile([128, 1], F32, tag="rstd", name="rstd")
        nc.vector.reciprocal(out=rstd, in_=std)
        y2 = y2_pool.tile([128, D], F32, tag="y2", name="y2")
        nc.vector.tensor_scalar(out=y2, in0=y1, scalar1=mv[:, 0:1],
                                scalar2=rstd, op0=ALU.subtract,
                                op1=ALU.mult)
        if apply_gb:
            nc.vector.tensor_mul(out=y2, in0=y2, in1=gb[0])
            nc.vector.tensor_add(out=y2, in0=y2, in1=gb[1])
        nc.sync.dma_start(out=y[qs, :], in_=y2)

    # ---- phase C: attention (nq outer; proj drip in pass 0, out-proj
    # drip in pass 1) ------------------------------------------------------
    with tc.tile_pool(name="ctxps", bufs=2, space="PSUM") as ctx_pool, \
         tc.tile_pool(name="sumps", bufs=2, space="PSUM") as sum_pool:
        drip = []
        for h in range(1, H):
            drip.extend(head_tasks(h))
        post = [lambda qc=qc: do_outproj(qc) for qc in range(4)]
        ctx_cur = sum_cur = None
        for i, (nq, h, cp) in enumerate(steps):
            while cursor[0] <= min(i + 2, len(steps) - 1):
                issue_scores()
            if cp == 0:
                ctx_cur = ctx_pool.tile([128, 512], F32, tag="ctx",
                                        name="ctx_ps")
                sum_cur = sum_pool.tile([1, 512], F32, tag="sum",
                                        name="sum_ps")
            pt = pt_q.pop(i)
            nc.tensor.matmul(ctx_cur, vp[cp][:, h], pt,
                             start=(cp == 0), stop=(cp == CP - 1),
                             perf_mode=DR)
            nc.tensor.matmul(sum_cur, ones2, pt,
                             start=(cp == 0), stop=(cp == CP - 1),
                             perf_mode=DR)
            if nq == 0 and drip:
                drip.pop(0)()
            elif nq == 1 and post and (i - CP * H) % 12 == 6:
                post.pop(0)()
            if cp == CP - 1:
                normalize(nq, h, ctx_cur, sum_cur)

    # ---- phase D: remaining out-projection q-chunks ----------------------
    while post:
        post.pop(0)()
    for qc in range(4, QC):
        do_outproj(qc)


def build(apply_gb=True):
    nc = bacc.Bacc("TRN2", target_bir_lowering=False, debug=False,
                   enable_asserts=False, num_devices=N_CORES)
    ap = {}
    ap['xtm'] = nc.dram_tensor("xtm", [NP, 128, 4, 2, 512], FP8,
                               kind="ExternalInput").ap()
    for nm in ('wq', 'wk'):
        ap[nm] = nc.dram_tensor(nm, [NP, 128, H, 2, 128], FP8,
                                kind="ExternalInput").ap()
    for nm in ('wv', 'wo'):
        ap[nm] = nc.dram_tensor(nm, [NP, 128, 2, 2, 512], FP8,
                                kind="ExternalInput").ap()
    ap['bq'] = nc.dram_tensor("bq", [128, H], F32, kind="ExternalInput").ap()
    ap['xq'] = nc.dram_tensor("xq", [TQ, D], F32, kind="ExternalInput").ap()
    ap['gamma'] = nc.dram_tensor("gamma", [D], F32,
                                 kind="ExternalInput").ap()
    ap['beta'] = nc.dram_tensor("beta", [D], F32, kind="ExternalInput").ap()
    ap['y'] = nc.dram_tensor("y", [TQ, D], F32, kind="ExternalOutput").ap()

    with tile.TileContext(nc) as tc, contextlib.ExitStack() as es:
        _body(nc, tc, ap, es, apply_gb)
    nc.compile()
    return nc


def _pack_pairs(w8, inner):
    """[D, N] fp8 -> [NP, 128, N//inner, 2, inner]: row (2p+j)*128+r,
    col (o*inner+m) lands at [p, r, o, j, m] (k-plane pairs contiguous)."""
    n = w8.shape[1]
    return np.ascontiguousarray(
        w8.reshape(NP, 2, 128, n // inner, inner).transpose(0, 2, 3, 1, 4))


def make_in_maps(inputs):
    """Per-core input maps; x token-rotated so q tokens come first."""
    f32 = {k: np.asarray(v, dtype=np.float32) for k, v in inputs.items()}

    def w8(nm):
        return (f32[nm] * WS).astype(F8)

    shared = {
        'wq': _pack_pairs(w8('Wq'), 128),
        'wk': _pack_pairs(w8('Wk'), 128),
        'wv': _pack_pairs(w8('Wv'), 512),
        'wo': _pack_pairs(w8('Wo'), 512),
        'bq': np.ascontiguousarray(
            (WS * f32['bq']).reshape(H, 128).T.astype(np.float32)),
        'gamma': f32['gamma'],
        'beta': f32['beta'],
    }
    resid_c = f32['bo'] + f32['bv'] @ f32['Wo']
    x = f32['x']
    in_maps = []
    for core in range(N_CORES):
        b, g = divmod(core, 2)
        xr = np.roll(x[b], -TQ * g, axis=0)
        xt8 = xr.T.astype(F8)  # [D, T]
        in_maps.append({
            'xtm': _pack_pairs(xt8, 512),
            'xq': np.ascontiguousarray(XQS * (xr[:TQ] + resid_c)),
            **shared})
    return in_maps


_NC = None
_NC_GB = None


def kernel(**inputs):
    global _NC, _NC_GB
    apply_gb = not (np.all(np.asarray(inputs['gamma']) == 1.0)
                    and np.all(np.asarray(inputs['beta']) == 0.0))
    if _NC is None or _NC_GB != apply_gb:
        _NC = build(apply_gb)
        _NC_GB = apply_gb
    in_maps = make_in_maps(inputs)
    res = bass_utils.run_bass_kernel_spmd(_NC, in_maps,
                                          core_ids=list(range(N_CORES)))
    out = np.empty((B, T, D), dtype=np.float32)
    for core in range(N_CORES):
        b, g = divmod(core, 2)
        out[b, TQ * g:TQ * (g + 1)] = res.results[core]['y']
    return out
